# revision 36
# baseline (speedup 1.0000x reference)
"""Trainium2 Bass kernel for causal self-attention (B=2, T=2048, C=1024, H=16).

Sharding: tensor-parallel over heads x data-parallel over batch.
Each of the 8 cores handles one (batch b, head-group g) pair: b = core // 4,
g = core % 4, where a head group is 4 consecutive heads (heads 4g..4g+3).

Per-core pipeline (v6 — deep software pipeline):
  *  Host pre-packs every DRAM input as the exact SBUF image; the first
     QKV chain is gated on a single ~830 KB DMA (cst | wk0 | xq0a), the
     second on one more (wq0 | xq0b), for a ~11.5 us first-chain start.
  *  PE warmup matmuls (tiny N=16, gated only on a DVE memset) run across
     the whole head-DMA window so HAM stays at 8/8.
  *  QKV projection chains and the output projection are generators that
     yield one matmul at a time; a global work queue pumps them into PE
     gaps inside the attention inner loop.
  *  Attention per head-pair in transposed layout S^T[k, q] = kT.T @ qT,
     two 128-k-blocks per iteration (batched S / PV matmul groups halve
     PE pipeline-refill costs), two heads as row-group-overlapped matmul
     pairs (base partitions 0/64) writing one [128,1024] two-bank PSUM
     tile, so exp on ACT covers both heads in one op (valid columns
     only); causal diagonal masked by a 0/1 multiply on DVE; PV
     (v_aug.T @ P^T, row 64 = softmax denominator) trails one iteration.
  *  Normalize: denominator row to SBUF on ACT, gpsimd partition
     broadcast, reciprocal + multiply on DVE.  The final q-chunk
     normalizes per 128-column piece so the output projection can start
     immediately (shorter tail), with its casts split DVE/ACT.
  *  Output projection partials are cast to bf16 and DMA'd per 128-row
     block; the TP all-reduce over the 4 head-groups runs on the host.
"""

import os
import numpy as np
from collections import deque
from contextlib import ExitStack

import concourse.bass as bass
import concourse.tile as tile
from concourse import bacc, library_config, mybir
from concourse.bass import ts
from concourse.bass_utils import run_bass_kernel_spmd

F32 = mybir.dt.float32
BF16 = mybir.dt.bfloat16
AF = mybir.ActivationFunctionType
PSUM = bass.MemorySpace.PSUM

B, T, C, H = 2, 2048, 1024, 16
HD = C // H              # 64
HPC = 4                  # heads per core
PAIRS = 2                # head pairs per core
CI = C // 128            # 8 contraction chunks
TB = T // 128            # 16 t-blocks
NQC = T // 512           # 4 q-chunks
N_CORES = 8

OUT_BF16 = os.environ.get("KBASS_OUT", "bf16") == "bf16"
OUT_DT = BF16 if OUT_BF16 else F32
WARMUP = int(os.environ.get("KBASS_WARMUP", "130"))
PUMP = int(os.environ.get("KBASS_PUMP", "4"))
GMASK = int(os.environ.get("KBASS_GMASK", "2"))  # 0=DVE 1=gpsimd 2=split
LDVE = os.environ.get("KBASS_LDVE", "1") == "1"

H0W = 192 + 2048 + 2048   # cst | wk0 | wq0 | xq0(ci 0-3)
H1W = 2048                # xq0(ci 4-7)


def _emit(tc, nc, h0_d, h1_d, xq_d, wkq1_d, wv_d, wp_d, out_d):
    ctx = ExitStack()
    with ctx:
        pers = ctx.enter_context(tc.tile_pool(name="pers", bufs=1))
        nc.gpsimd.load_library(library_config.attn)

        # ---------------- persistent SBUF ----------------
        h0 = pers.tile([128, H0W], BF16, name="h0")
        h1 = pers.tile([128, H1W], BF16, name="h1")
        xq_t = [None] + [pers.tile([128, CI * 512], BF16, name=f"xq{qc}")
                         for qc in range(1, NQC)]
        wkq1 = pers.tile([128, 2048], BF16, name="wkq1")
        wv_sb = pers.tile([128, 2048], BF16, name="wv")
        wp_sb = pers.tile([128, 2048], BF16, name="wp")
        qT = [pers.tile([128, T], BF16, name=f"qT{p}") for p in range(PAIRS)]
        kT = [pers.tile([128, T], BF16, name=f"kT{p}") for p in range(PAIRS)]
        # v_all[(h, tb)] block = [128 k, 65]; col 64 = 1.0 (softmax denominator)
        v_all = pers.tile([128, HPC * TB * 65], BF16, name="v_all")
        yT = [pers.tile([128, T], BF16, name=f"yT{p}") for p in range(PAIRS)]

        cst = h0[:, 0:192]
        mask_d = h0[:, 0:128]

        def xchunk(qc, ci):
            """[128, 512] chunk of x^T: partitions = C-chunk ci, cols = t."""
            if qc == 0:
                if ci < 4:
                    return h0[:, 2240 + ci * 512: 2240 + (ci + 1) * 512]
                return h1[:, (ci - 4) * 512: (ci - 3) * 512]
            return xq_t[qc][:, ts(ci, 512)]

        def wslice(pair, kq):
            """[128, 1024] packed lhsT chunks for k (kq=0) / q (kq=1)."""
            if pair == 0:
                return h0[:, 192 + kq * 1024: 192 + (kq + 1) * 1024]
            return wkq1[:, kq * 1024:(kq + 1) * 1024]

        # warmup weights live in SBUF with no DMA dependency (DVE memset)
        warm_sb = pers.tile([128, 16], BF16, name="warm")
        nc.vector.memset(warm_sb[:], 0.0)

        # ---------------- head DMAs (HWDGE FIFO on sync: order = priority) --
        nc.sync.dma_start(h0[:], h0_d[:])
        nc.sync.dma_start(h1[:], h1_d[:])
        nc.sync.dma_start(wv_sb[:], wv_d[:])
        nc.sync.dma_start(xq_t[1][:], xq_d[0])
        nc.sync.dma_start(wkq1[:], wkq1_d[:])
        nc.sync.dma_start(xq_t[2][:], xq_d[1])
        nc.sync.dma_start(xq_t[3][:], xq_d[2])
        nc.sync.dma_start(wp_sb[:], wp_d[:])

        v4 = v_all[:].rearrange("p (h t c) -> p h t c", h=HPC, c=65)
        for h in range(HPC):
            nc.vector.tensor_copy(
                v4[:, h, :, 64:65],
                cst[:, 128:129].unsqueeze(1).broadcast_to([128, TB, 1]),
            )

        # ---------------- work queue of single-matmul generators ----------
        work = deque()
        finished = [0]

        def wrap(g, flag=None):
            yield from g
            finished[0] += 1
            if flag is not None:
                flag[0] = True

        def pump(n):
            while n > 0 and work:
                try:
                    next(work[0])
                    n -= 1
                except StopIteration:
                    work.popleft()

        def drain_chains(k):
            while finished[0] < k and work:
                try:
                    next(work[0])
                except StopIteration:
                    work.popleft()

        def drain_flag(flag):
            while not flag[0] and work:
                try:
                    next(work[0])
                except StopIteration:
                    work.popleft()

        with (
            tc.tile_pool(name="psS", bufs=1, space=PSUM) as psS,
            tc.tile_pool(name="psY", bufs=1, space=PSUM) as psY,
            tc.tile_pool(name="pP", bufs=6) as pP,
            tc.tile_pool(name="pN", bufs=2) as pN,
        ):
            def normalize(p, qc, ypt, hh, c0, c1, tail=False):
                # yT[head rows, qc cols c0:c1] = ypt[0:64, c0:c1] / l[c0:c1]
                off = hh * 64
                w = c1 - c0
                l_sb = pN.tile([1, 512], F32, tag="l1")
                if LDVE:
                    nc.vector.tensor_copy(l_sb[:, 0:w], ypt[hh][64:65, c0:c1])
                else:
                    nc.scalar.copy(l_sb[:, 0:w], ypt[hh][64:65, c0:c1])
                rl1 = pN.tile([1, 512], F32, tag="rl1")
                nc.vector.reciprocal_approx_fast(rl1[:, 0:w], l_sb[:, 0:w])
                lb = pN.tile([64, 512], F32, tag="lb")
                nc.gpsimd.partition_broadcast(lb[:, 0:w], rl1[:, 0:w])
                nc.vector.tensor_mul(
                    yT[p][off:off + 64, qc * 512 + c0: qc * 512 + c1],
                    ypt[hh][0:64, c0:c1], lb[:, 0:w],
                )

            # ------- attention for one head pair: one flat pipeline over all
            # (qc, 2-k-block batch) steps; PV trails S by one batch, incl.
            # across qc boundaries, so the S stream never stalls there -----
            def attn_pair(p, at_qc_start=None, at_qc_end=None,
                          fine_last=False, pump_n=PUMP):
                prev = None  # (qc, kbs, pts_b, ypt, is_last_of_qc)

                def emit_pv(qc, kbs, pts_b, ypt, _last):
                    nkb = 4 * qc + 4
                    for kb, pt in zip(kbs, pts_b):
                        col = max(0, (kb - 4 * qc) * 128)
                        for hh in (0, 1):
                            nc.tensor.matmul(
                                ypt[hh][0:65, col:512],
                                v4[:, 2 * p + hh, kb, :],
                                pt[:, hh * 512 + col:(hh + 1) * 512],
                                start=(kb == 0), stop=(kb == nkb - 1),
                            )

                def finish_prev():
                    if prev is None:
                        return
                    emit_pv(*prev)
                    if prev[4]:  # last batch of its qc -> normalize
                        pqc, pypt = prev[0], prev[3]
                        fine = fine_last and pqc == NQC - 1
                        for c0 in (range(0, 512, 128) if fine else (0,)):
                            c1 = c0 + (128 if fine else 512)
                            for hh in (0, 1):
                                normalize(p, pqc, pypt, hh, c0, c1, tail=fine)
                        if at_qc_end is not None:
                            at_qc_end(pqc)

                for qc in range(NQC):
                    if at_qc_start is not None:
                        at_qc_start(qc)
                    ypt = [psY.tile([128, 512], F32, tag=f"ypt{hh}", bufs=1,
                                    name=f"y{p}q{qc}h{hh}") for hh in (0, 1)]
                    nkb = 4 * qc + 4
                    for kbp in range(nkb // 2):
                        kbs = (2 * kbp, 2 * kbp + 1)
                        sps_b, pts_b = [], []
                        for kb in kbs:
                            col = max(0, (kb - 4 * qc) * 128)
                            # both heads in one 2-bank PSUM tile (hh0:
                            # 0:512, hh1: 512:1024) -> one exp for both
                            sp = psS.tile([128, 1024], F32, tag="sp", bufs=2,
                                          name="sp")
                            for hh in (0, 1):
                                off = hh * 64
                                nc.tensor.matmul(
                                    sp[:, hh * 512 + col:(hh + 1) * 512],
                                    kT[p][off:off + 64, ts(kb, 128)],
                                    qT[p][off:off + 64,
                                          qc * 512 + col:(qc + 1) * 512],
                                    start=True, stop=True,
                                )
                            sps_b.append(sp)
                        for kb, sp in zip(kbs, sps_b):
                            col = max(0, (kb - 4 * qc) * 128)
                            pt = pP.tile([128, 1024], BF16, tag="pt",
                                         name="pt")
                            if col == 0:
                                nc.scalar.activation(pt[:], sp[:], AF.Exp)
                            else:
                                for hh in (0, 1):
                                    nc.scalar.activation(
                                        pt[:, hh * 512 + col:(hh + 1) * 512],
                                        sp[:, hh * 512 + col:(hh + 1) * 512],
                                        AF.Exp)
                            if kb >= 4 * qc:  # mask the diagonal 128-block
                                for hh in (0, 1):
                                    reg = pt[:, hh * 512 + col:
                                             hh * 512 + col + 128]
                                    gps = GMASK == 1 or (GMASK == 2 and hh == 0)
                                    if gps:  # keep where q - k >= 0
                                        nc.gpsimd.affine_select(
                                            out=reg, in_=reg,
                                            compare_op=mybir.AluOpType.is_ge,
                                            fill=0.0, base=0,
                                            channel_multiplier=-1,
                                            pattern=[[1, 128]],
                                        )
                                    else:
                                        nc.vector.tensor_mul(reg, reg, mask_d)
                            pts_b.append(pt)
                        pump(pump_n)
                        finish_prev()
                        prev = (qc, kbs, pts_b, ypt, kbp == nkb // 2 - 1)
                finish_prev()

            # ---------------- phase A: QKV chains + attention pair 0 -------
            with tc.tile_pool(name="psA", bufs=2, space=PSUM) as psA:
                # PE warmup across the head DMA window (keeps HAM at 8/8);
                # gated only on a DVE memset, not on any DMA
                wps = psA.tile([16, 16], F32, tag="psqk", name="wps")
                for _ in range(WARMUP):
                    nc.tensor.matmul(wps[:], warm_sb[:], warm_sb[:],
                                     start=True, stop=True)

                def gen_qk(pair, kq, qc, pool=None, tag="psqk"):
                    w = wslice(pair, kq)
                    dst = (kT if kq == 0 else qT)[pair]
                    ps = (pool or psA).tile([128, 512], F32, tag=tag,
                                            name=f"qk{pair}{kq}{qc}")
                    for ci in range(CI):
                        nc.tensor.matmul(
                            ps[:], w[:, ts(ci, 128)], xchunk(qc, ci),
                            start=(ci == 0), stop=(ci == CI - 1),
                        )
                        yield
                    nc.vector.tensor_copy(dst[:, ts(qc, 512)], ps[:])

                def gen_v(tb):
                    qc, j = tb // 4, tb % 4
                    psv = psA.tile([128, 256], F32, tag="psqk", name=f"v{tb}")
                    for ci in range(CI):
                        nc.tensor.matmul(
                            psv[:],
                            xchunk(qc, ci)[:, j * 128:(j + 1) * 128],
                            wv_sb[:, ts(ci, 256)],
                            start=(ci == 0), stop=(ci == CI - 1),
                        )
                        yield
                    nc.vector.tensor_copy(
                        v4[:, :, tb, 0:64],
                        psv[:].rearrange("p (h c) -> p h c", c=64),
                    )

                def gen_head_qk():
                    # k0(0)/q0(0) interleaved at half-chain granularity so
                    # the ci 0-3 matmuls (gated on h0 only) run while the
                    # h1 DMA (xq0 ci 4-7) is still in flight
                    wk, wq = wslice(0, 0), wslice(0, 1)
                    psk = psA.tile([128, 512], F32, tag="psqk", name="qkh0")
                    psq = psA.tile([128, 512], F32, tag="psqk", name="qkh1")
                    for ps, w in ((psk, wk), (psq, wq)):
                        for ci in range(4):
                            nc.tensor.matmul(
                                ps[:], w[:, ts(ci, 128)], xchunk(0, ci),
                                start=(ci == 0), stop=False)
                            yield
                    for ps, w, dst in ((psk, wk, kT[0]), (psq, wq, qT[0])):
                        for ci in range(4, CI):
                            nc.tensor.matmul(
                                ps[:], w[:, ts(ci, 128)], xchunk(0, ci),
                                start=False, stop=(ci == CI - 1))
                            yield
                        nc.vector.tensor_copy(dst[:, 0:512], ps[:])

                def grp(qc):
                    if qc == 0:
                        return ([wrap(gen_head_qk())]
                                + [wrap(gen_v(tb)) for tb in range(4)])
                    return ([wrap(gen_qk(0, 0, qc)), wrap(gen_qk(0, 1, qc))]
                            + [wrap(gen_v(tb)) for tb in range(4 * qc, 4 * qc + 4)])

                # pair-1 chains for qc >= 1 are reserved as phase-B
                # boundary filler (they only depend on long-landed DMAs)
                for qc in range(NQC):
                    work.extend(grp(qc))
                work.append(wrap(gen_qk(1, 0, 0)))
                work.append(wrap(gen_qk(1, 1, 0)))

                attn_pair(0, at_qc_start=lambda qc: drain_chains(5 + 6 * qc))
                drain_chains(6 * NQC + 1)  # everything queued in phase A

            # ---------------- phase B: attention pair 1 + projection -------
            with (
                tc.tile_pool(name="psO", bufs=2, space=PSUM) as psO,
                tc.tile_pool(name="pO", bufs=3) as pO,
            ):
                def gen_proj(tb, last=False):
                    ot = pO.tile([128, 1024], OUT_DT, tag="ot", name=f"ot{tb}")
                    for cc in (0, 1):
                        po = psO.tile([128, 512], F32, tag="po",
                                      name=f"po{tb}{cc}")
                        for p in range(PAIRS):
                            nc.tensor.matmul(
                                po[:], yT[p][:, ts(tb, 128)],
                                wp_sb[:, p * 1024 + cc * 512:
                                      p * 1024 + cc * 512 + 512],
                                start=(p == 0), stop=(p == PAIRS - 1),
                            )
                            yield
                        if last and cc == 1:
                            # tail: split casts across engines (ACT is idle)
                            nc.scalar.copy(ot[:, ts(cc, 512)], po[:])
                        else:
                            nc.vector.tensor_copy(ot[:, ts(cc, 512)], po[:])
                    nc.sync.dma_start(out_d[ts(tb, 128), :], ot[:])

                qflag = {}

                def b_start(qc):
                    if qc >= 1:  # qk1(qc) chains must be emitted before use
                        drain_flag(qflag[qc])
                    if qc < NQC - 1:  # reserved pair-1 chains for qc+1
                        qflag[qc + 1] = [False]
                        work.append(wrap(gen_qk(1, 0, qc + 1, psO, "po")))
                        work.append(wrap(gen_qk(1, 1, qc + 1, psO, "po"),
                                         qflag[qc + 1]))

                def b_end(qc):
                    work.extend(wrap(gen_proj(tb, last=(qc == NQC - 1)))
                                for tb in range(4 * qc, 4 * qc + 4))

                attn_pair(1, at_qc_start=b_start, at_qc_end=b_end,
                          fine_last=True)
                drain_chains(finished[0] + len(work))


_NC_CACHE = None


def _build():
    global _NC_CACHE
    if _NC_CACHE is not None:
        return _NC_CACHE
    nc = bacc.Bacc("TRN2", target_bir_lowering=False, debug=False,
                   num_devices=N_CORES)
    h0_d = nc.dram_tensor("h0", [128, H0W], BF16, kind="ExternalInput")
    h1_d = nc.dram_tensor("h1", [128, H1W], BF16, kind="ExternalInput")
    xq_d = nc.dram_tensor("xq", [NQC - 1, 128, CI * 512], BF16,
                          kind="ExternalInput")
    wkq1_d = nc.dram_tensor("wkq1", [128, 2048], BF16, kind="ExternalInput")
    wv_d = nc.dram_tensor("wv", [128, 2048], BF16, kind="ExternalInput")
    wp_d = nc.dram_tensor("wp", [128, 2048], BF16, kind="ExternalInput")
    out_d = nc.dram_tensor("out", [T, C], OUT_DT, kind="ExternalOutput")

    with tile.TileContext(nc) as tc:
        _emit(tc, nc, h0_d, h1_d, xq_d, wkq1_d, wv_d, wp_d, out_d)
    nc.compile()
    _NC_CACHE = nc
    return nc


def _pack_pair(m):
    # [1024, 128] -> lhsT chunks layout [128, 8*128]
    return np.ascontiguousarray(
        m.reshape(CI, 128, 128).transpose(1, 0, 2).reshape(128, 1024))


def _bf16(a):
    import ml_dtypes
    return np.ascontiguousarray(a.astype(ml_dtypes.bfloat16))


def _cst():
    cst = np.zeros((128, 192), dtype=np.float32)
    # mask[k, q] = 1 where q >= k
    cst[:, 0:128] = np.triu(np.ones((128, 128), dtype=np.float32))
    cst[:, 128] = 1.0
    return cst


def _in_maps(x, w_attn, w_proj):
    x = np.asarray(x, dtype=np.float32)
    w_attn = np.asarray(w_attn, dtype=np.float32)
    w_proj = np.asarray(w_proj, dtype=np.float32)
    # xq[qc][p, ci*512+col] = x[b][qc*512+col, ci*128+p]
    xq_b = []
    for b in range(B):
        xT = x[b].T.reshape(CI, 128, T)  # [ci, p, t]
        xq_b.append(np.stack([
            xT[:, :, qc * 512:(qc + 1) * 512].transpose(1, 0, 2).reshape(128, CI * 512)
            for qc in range(NQC)]))
    cst = _cst()
    maps = []
    for core in range(N_CORES):
        b, g = core // HPC, core % HPC
        cols = slice(g * 256, (g + 1) * 256)
        wk_full = w_attn[:, 0 * C:1 * C][:, cols]
        wq_full = w_attn[:, 1 * C:2 * C][:, cols] * np.float32(1.0 / np.sqrt(HD))
        wv_full = w_attn[:, 2 * C:3 * C][:, cols]
        wk_p = [_pack_pair(wk_full[:, p * 128:(p + 1) * 128]) for p in range(PAIRS)]
        wq_p = [_pack_pair(wq_full[:, p * 128:(p + 1) * 128]) for p in range(PAIRS)]
        h0 = np.concatenate([cst, wk_p[0], wq_p[0],
                             xq_b[b][0][:, 0:CI * 256]], axis=1)
        h1 = xq_b[b][0][:, CI * 256:]
        wkq1_in = np.concatenate([wk_p[1], wq_p[1]], axis=1)
        wv_in = wv_full.reshape(CI, 128, 256).transpose(1, 0, 2).reshape(128, 2048)
        wp_in = (w_proj[g * 256:(g + 1) * 256, :]
                 .reshape(PAIRS, 128, 1024).transpose(1, 0, 2).reshape(128, 2048))
        maps.append({"h0": _bf16(h0), "h1": _bf16(h1),
                     "xq": _bf16(xq_b[b][1:]), "wkq1": _bf16(wkq1_in),
                     "wv": _bf16(wv_in), "wp": _bf16(wp_in)})
    return maps


def _assemble(results, b_proj):
    b_proj = np.asarray(b_proj, dtype=np.float32)
    out = np.zeros((B, T, C), dtype=np.float32)
    for core in range(N_CORES):
        out[core // HPC] += np.asarray(results[core]["out"], dtype=np.float32)
    out += b_proj[None, None, :]
    return out


def kernel(x, w_attn, w_proj, b_proj):
    nc = _build()
    maps = _in_maps(x, w_attn, w_proj)
    res = run_bass_kernel_spmd(nc, maps, list(range(N_CORES)))
    return _assemble(res.results, b_proj)


def kernel_traced(x, w_attn, w_proj, b_proj):
    """Like kernel() but with NTFF tracing; returns (out, BassKernelResults)."""
    nc = _build()
    maps = _in_maps(x, w_attn, w_proj)
    res = run_bass_kernel_spmd(nc, maps, list(range(N_CORES)), trace=True)
    return _assemble(res.results, b_proj), res
